# revision 1
# baseline (speedup 1.0000x reference)
"""Trainium2 Bass kernel for nn_DiagonalFunc (64 parallel 2-layer MLPs).

Computation (per batch row b, branch i):
    inp  = concat(x[b, i], z[b, :])                       # 65 features
    h    = inp @ W1[i] + b1[i]                            # [256]
    out  = sum(elu(h) * W2[i]) + b2[i]                    # scalar

Mapping (per core, batch-sharded 8192/8 = 1024 rows):
  - Layer 1 on TensorE as f32r matmuls: stationary = per-(branch, hidden-chunk)
    [128 x 128] weight block (rows 0-63: z-part of W1, row 64+i: x-row of W1,
    rest zero); moving = shared ZX tile [128 rows = z^T | x^T, 512 batch cols].
    PSUM tile [128 hidden, 1024] holds both 512-batch halves of one
    (branch, chunk).
  - ScalarE drains PSUM with Exp(+b1 bias) -> e in SBUF.
  - VectorE runs a custom fused DVE op: u = relu(h+b1) + min(e,1) - 1 = elu(h+b1)
    in ONE 1x pass over PSUM.
  - Layer 2 on TensorE: fp32 matmuls, stationary = W2 column [128,1], moving = u
    [128, 512]; M=1 output lands on psum partition 32*j via col-tiling
    (tile_position), 4 branches per psum bank group; hc chunks accumulate.
    The 8 matmuls of one branch-group are emitted back-to-back so different
    col-groups overlap in the PE array.
  - ScalarE drains the [128, 1024] group-output psum with Identity(+b2 bias);
    DMA gathers rows {0,32,64,96} to the DRAM output.
"""
import numpy as np

import concourse.bacc as bacc
import concourse.tile as tile
from concourse import mybir
from concourse.bass_utils import run_bass_kernel_spmd
import concourse.dve_ops as dve_ops
from concourse.dve_spec import Spec, Src0, Src1, C0, One, relu, minn
from concourse.dve_spec import lower as dve_lower, _has_src1
from concourse.dve_uop import DveOpSpec

# ---------------- problem constants (hardcoded per contract) ----------------
N_CORES = 8
BATCH = 8192
N_BR = 64          # branches
IN_F = 65          # per-branch input features (1 x + 64 z)
HID = 256          # hidden units -> 2 chunks of 128
B_CORE = BATCH // N_CORES   # 1024
F32 = mybir.dt.float32
F32R = mybir.dt.float32r
F16 = mybir.dt.float16

# ---------------- custom DVE op: elu from (h, exp(h+b1)) ----------------
def _elu_ref(in0, in1, s0, s1, imm2):
    h = in0.astype(np.float32) + s0
    return (np.maximum(h, 0) + np.minimum(in1.astype(np.float32), 1.0)
            - 1.0).astype(np.float32)


def _register_elu_op():
    name = "ELU_FE_ANT"
    if name in dve_ops._SUB_OPCODE_FOR_NAME:
        for op in dve_ops.OPS:
            if op.name == name:
                return op
    spec = Spec(body=relu(Src0 + C0) + minn(Src1, One) - One, reference=_elu_ref)
    opcode = max(dve_ops._SUB_OPCODE_FOR_NAME.values()) + 1
    assert opcode < 0x20
    shas = {}
    for ver in ("v3", "v4"):
        try:
            probe = DveOpSpec(name=name, opcode=opcode,
                              uops=dve_lower(spec, ver=ver),
                              rd1_en=_has_src1(spec))
            shas[ver] = probe.sha(ver)
        except Exception:
            pass
    op = dve_ops.DveOp(name, spec, subdim=False, uops_sha=shas)
    dve_ops.OPS.append(op)
    dve_ops.CUSTOM_DVE_SPECS[name] = spec
    dve_ops._SUB_OPCODE_FOR_NAME[name] = opcode
    return op


ELU_OP = _register_elu_op()

# ---------------- program build (cached) ----------------
_NC_CACHE = {}


def _build_nc(loop_n=1):
    if loop_n in _NC_CACHE:
        return _NC_CACHE[loop_n]
    nc = bacc.Bacc("TRN2", target_bir_lowering=False, debug=False,
                   num_devices=N_CORES)
    zx_d = nc.dram_tensor("zx", [128, B_CORE], F32R, kind="ExternalInput").ap()
    wst_d = nc.dram_tensor("wst", [128, N_BR * 2 * 128], F32R,
                           kind="ExternalInput").ap()
    b1_d = nc.dram_tensor("b1t", [128, N_BR * 2], F32, kind="ExternalInput").ap()
    w2_d = nc.dram_tensor("w2t", [128, N_BR * 2], F16, kind="ExternalInput").ap()
    b2_d = nc.dram_tensor("b2t", [128, 16], F32, kind="ExternalInput").ap()
    out_d = nc.dram_tensor("out", [B_CORE, N_BR], F32, kind="ExternalOutput").ap()

    Exp = mybir.ActivationFunctionType.Exp
    Ident = mybir.ActivationFunctionType.Identity

    with tile.TileContext(nc) as tc:
        with tc.tile_pool(name="const", bufs=1) as constp, \
             tc.tile_pool(name="wst", bufs=16) as wstp, \
             tc.tile_pool(name="epool", bufs=4) as epool, \
             tc.tile_pool(name="upool", bufs=20) as upool, \
             tc.tile_pool(name="osb", bufs=4) as osbp, \
             tc.tile_pool(name="psL1", bufs=3, space="PSUM") as psL1, \
             tc.tile_pool(name="psOut", bufs=2, space="PSUM") as psOut:

            zx = constp.tile([128, B_CORE], F32R, tag="zx")
            b1 = constp.tile([128, N_BR * 2], F32, tag="b1")
            w2 = constp.tile([128, N_BR * 2], F16, tag="w2")
            b2 = constp.tile([128, 16], F32, tag="b2")
            # One DMA queue (splitting across engine queues measured 6x
            # slower). Order so the first unit's dependencies land first:
            # zx half 0, group-0 weights, b1 bias, then the rest.
            wst_tiles = [wstp.tile([128, 1024], F32R, tag="wst",
                                   name=f"wst{g}") for g in range(16)]
            nc.sync.dma_start(zx[:, 0:512], zx_d[:, 0:512])
            nc.sync.dma_start(wst_tiles[0][:, 0:256], wst_d[:, 0:256])
            nc.sync.dma_start(b1[:], b1_d[:])
            nc.sync.dma_start(zx[:, 512:1024], zx_d[:, 512:1024])
            nc.sync.dma_start(wst_tiles[0][:, 256:1024], wst_d[:, 256:1024])
            nc.sync.dma_start(w2[:], w2_d[:])
            nc.sync.dma_start(b2[:], b2_d[:])
            for g in range(1, 16):
                nc.sync.dma_start(wst_tiles[g][:],
                                  wst_d[:, 1024 * g:1024 * (g + 1)])

            def emit_l2_and_drain(g, us):
                """Layer-2 cluster + psum-out drain + output DMA for group g.

                Emitted one group late so the PE stream never blocks the
                L1 matmuls that feed ACT/DVE (the critical chain).
                """
                for bc in range(2):
                    pout = psOut.tile([128, 512], F32, tag="pout")
                    for hc in range(2):
                        for j in range(4):
                            jc = 2 * (4 * g + j) + hc
                            u = us[(j, hc)]
                            nc.tensor.matmul(
                                pout[32 * j:32 * j + 1, :],
                                w2[:, jc:jc + 1],
                                u[:, 512 * bc:512 * (bc + 1)],
                                start=(hc == 0), stop=(hc == 1),
                                tile_position=(0, 32 * j))
                    osb = osbp.tile([128, 512], F32, tag="osb")
                    nc.scalar.activation(osb[:], pout[:], Ident,
                                         bias=b2[:, g:g + 1])
                    for j in range(4):
                        br = 4 * g + j
                        nc.sync.dma_start(
                            out_d[512 * bc:512 * (bc + 1), br:br + 1],
                            osb[32 * j:32 * j + 1, :])

            def body(_iv=None):
                pending = None
                for g in range(16):
                    wg = wst_tiles[g]
                    us = {}
                    for j in range(4):
                        br = 4 * g + j
                        for hc in range(2):
                            jc = 2 * br + hc
                            loc = (2 * j + hc) * 128  # col offset inside wg
                            P = psL1.tile([128, 1024], F32, tag="psl1")
                            nc.tensor.matmul(P[:, 0:512], wg[:, loc:loc + 128],
                                             zx[:, 0:512], start=True, stop=True)
                            nc.tensor.matmul(P[:, 512:1024],
                                             wg[:, loc:loc + 128],
                                             zx[:, 512:1024],
                                             start=True, stop=True)
                            e = epool.tile([128, 1024], F32, tag="e")
                            nc.scalar.activation(e[:], P[:], Exp,
                                                 bias=b1[:, jc:jc + 1])
                            u = upool.tile([128, 1024], F16, tag="u")
                            nc.vector._custom_dve(ELU_OP, out=u[:], in0=P[:],
                                                  in1=e[:], s0=b1[:, jc:jc + 1])
                            us[(j, hc)] = u
                    if pending is not None:
                        emit_l2_and_drain(*pending)
                    pending = (g, us)
                emit_l2_and_drain(*pending)

            if isinstance(loop_n, tuple):
                n_iter, n_body = loop_n
            else:
                n_iter, n_body = loop_n, 1
            if n_iter == 1:
                for _ in range(n_body):
                    body()
            else:
                with tc.For_i(0, n_iter, 1):
                    for _ in range(n_body):
                        body()
    nc.compile()
    _NC_CACHE[loop_n] = nc
    return nc


# ---------------- host-side prep + entry point ----------------
def _prep_shared(W1, b1, W2, b2):
    """Host-side rearrangement of the (replicated) weights."""
    W1 = np.asarray(W1, dtype=np.float32)
    b1 = np.asarray(b1, dtype=np.float32)
    W2 = np.asarray(W2, dtype=np.float32)
    b2 = np.asarray(b2, dtype=np.float32)
    # wst [128 rows, 64br * 2hc * 128] ; col-block index = br*2 + hc ordered
    # within groups: block (g, j, hc) lives at 1024*g + (2*j+hc)*128
    wst = np.zeros((128, N_BR * 2 * 128), dtype=np.float32)
    b1t = np.zeros((128, N_BR * 2), dtype=np.float32)
    w2t = np.zeros((128, N_BR * 2), dtype=np.float16)
    for br in range(N_BR):
        g, j = divmod(br, 4)
        for hc in range(2):
            off = 1024 * g + (2 * j + hc) * 128
            wst[0:64, off:off + 128] = W1[br, 1:65, 128 * hc:128 * (hc + 1)]
            wst[64 + br, off:off + 128] = W1[br, 0, 128 * hc:128 * (hc + 1)]
            jc = 2 * br + hc
            b1t[:, jc] = b1[br, 128 * hc:128 * (hc + 1)]
            w2t[:, jc] = W2[br, 128 * hc:128 * (hc + 1)]
    b2t = np.zeros((128, 16), dtype=np.float32)
    for g in range(16):
        for j in range(4):
            b2t[32 * j, g] = b2[4 * g + j]
    return wst, b1t, w2t, b2t


def kernel(x, z, W1, b1, W2, b2):
    x = np.asarray(x, dtype=np.float32)
    z = np.asarray(z, dtype=np.float32)
    wst, b1t, w2t, b2t = _prep_shared(W1, b1, W2, b2)
    nc = _build_nc()
    in_maps = []
    for c in range(N_CORES):
        sl = slice(c * B_CORE, (c + 1) * B_CORE)
        zx = np.concatenate([z[sl].T, x[sl].T], axis=0).astype(np.float32)
        zx = np.ascontiguousarray(zx)
        in_maps.append({"zx": zx, "wst": wst, "b1t": b1t, "w2t": w2t,
                        "b2t": b2t})
    res = run_bass_kernel_spmd(nc, in_maps, list(range(N_CORES)))
    out = np.concatenate([res.results[c]["out"] for c in range(N_CORES)],
                         axis=0)
    return out.astype(np.float32)



# revision 17
# speedup vs baseline: 1.4423x; 1.4423x over previous
"""Trainium2 Bass kernel for nn_DiagonalFunc (64 parallel 2-layer MLPs).

Computation (per batch row b, branch i):
    h'   = concat(x[b,i], z[b,:]) @ W1[i] + b1[i]          # [256]
    out  = sum(elu(h') * W2[i]) + b2[i]                    # scalar

Structure: the linear term sum(w2*h') of every branch collapses into a
host-precomputed W1@W2 matmul (exact, f32r), so the engines only compute
a per-element residual. Hidden units are permuted per-branch by |w2|
descending; chunk 0 holds the 128 largest-|w2| units, chunk 1 the 128
smallest. Three per-tile pipes (no Pool: GPSIMD cannot read PSUM; no Exp:
Softplus and Exp live in different ACT table sets):

  S  (all chunk-1 tiles): ACT v = Silu(c - b*h') in one pass;
      elu(x) ~ a*silu(c - b*x) + x + e  (a folded into the L2 weights,
      x into the linear matmul, e into the consts). Fit error lands on
      the smallest-|w2| units only -> ~4e-3 output error.
  D' (chunk-0): ACT t = Relu(-h') fp16; DVE v = t + tc*(c1+c2*tc+c3*tc^2)
      with tc = min(t, T): cubic model of psi(t) = elu - h' = t-1+e^{-t}.
  DD (chunk-0): DVE t = (h' * -1) max 0 (stock tensor_scalar from PSUM);
      DVE v = same cubic. No ACT at all - balances the ACT/DVE load.

Layer 1 on TensorE (f32r): stationary per (branch, chunk) [128x128]
(rows 0-63 z-weights, row 64 = b1 fed by a ones-row in the moving tile,
row 65+br = x-weights); moving = shared zx tile [128, 1024 batch], so
PSUM holds h' = h + b1 directly. Branch 63's x-row lives in a second
moving tile zxb (128-row budget: 64 z + 1 ones + 63 x).

Layer 2 on TensorE (fp16): per tile one matmul, stationary [128, 64]
with only column br nonzero, all branches accumulating into a single
PSUM tile [64 branches, 1024 batch] together with the linear W1@W2
matmuls (f32r, with per-branch consts riding the ones-row). ScalarE
drains once; DMA writes [64, 1024]; host transposes.

Engine balance per core (est): PE 110us, ACT 109us, DVE 110us.
"""
import numpy as np

import concourse.bacc as bacc
import concourse.tile as tile
from concourse import mybir
from concourse.bass_utils import run_bass_kernel_spmd
import concourse.dve_ops as dve_ops
from concourse.dve_spec import (Spec, Src0, Src1, C0, C1, C2, C3, One,
                                relu, sq, minn, lower as dve_lower,
                                _has_src1, _spill_c3_to_src1)
from concourse.dve_uop import DveOpSpec

# ---------------- problem constants (hardcoded per contract) ----------------
N_CORES = 8
BATCH = 8192
N_BR = 64
IN_F = 65
HID = 256
B_CORE = BATCH // N_CORES   # 1024
F32 = mybir.dt.float32
F32R = mybir.dt.float32r
F16 = mybir.dt.float16

# PSI cubic (D'/DD pipes): g(t)=e^{-t}-1 ~ c1*t+c2*t^2+c3*t^3 on [0,T],
# density-weighted fit; exact linear tail beyond T.
PSI_T = 3.25
PSI_C1, PSI_C2, PSI_C3 = -0.946418, 0.360178, -0.050623

# S pipe silu fit: elu(x) ~ SP_A*silu(SP_C - SP_B*x) + x + SP_E
# (Softplus is unavailable in the act tables; Silu shares a table with
# Relu/Identity so the whole kernel uses one table load.)
SP_A = 0.6278981343517278
SP_B = 1.2817224719245803
SP_C = -0.7297317049541422
SP_E = 0.14582581857025065

# chunk-0 (large |w2|) tile pipes: weave of D' and DD
_C0_COUNTS = {"D": 38, "DD": 26}


def _weave(counts):
    rem = dict(counts)
    pat = []
    for _ in range(sum(counts.values())):
        k = max(rem, key=lambda p: (rem[p] / counts[p], p))
        pat.append(k)
        rem[k] -= 1
    return pat


_C0 = _weave(_C0_COUNTS)
# tile index blk = 2*br + hc: even -> chunk0 (D'/DD), odd -> chunk1 (S)
PIPES = []
for _br in range(N_BR):
    PIPES.append(_C0[_br])
    PIPES.append("S")

# ---------------- custom DVE op ----------------
def _psi_ref(in0, in1, s0, s1, imm2):
    t = in0.astype(np.float32)
    T = in1.astype(np.float32)
    tc = np.minimum(t, T)
    return (t + tc * (s0 + s1 * tc + imm2 * tc * tc)).astype(np.float32)


def _register_op(name, body, ref):
    if name in dve_ops._SUB_OPCODE_FOR_NAME:
        for op in dve_ops.OPS:
            if op.name == name:
                return op
    spec = Spec(body=body, reference=ref)
    opcode = max(dve_ops._SUB_OPCODE_FOR_NAME.values()) + 1
    assert opcode < 0x20
    shas = {}
    for ver in ("v3", "v4"):
        try:
            probe = DveOpSpec(name=name, opcode=opcode,
                              uops=dve_lower(spec, ver=ver),
                              rd1_en=_has_src1(spec))
            shas[ver] = probe.sha(ver)
        except Exception:
            pass
    op = dve_ops.DveOp(name, spec, subdim=False, uops_sha=shas)
    dve_ops.OPS.append(op)
    dve_ops.CUSTOM_DVE_SPECS[name] = spec
    dve_ops._SUB_OPCODE_FOR_NAME[name] = opcode
    return op


_tc = minn(Src0, C3)
PSI3_OP = _register_op(
    "PSI3_ANT",
    _spill_c3_to_src1(Src0 + _tc * (C0 + C1 * _tc + C2 * sq(_tc))),
    _psi_ref)

# ---------------- program build (cached) ----------------
_NC_CACHE = {}

L2_LAG = 5   # tiles of L2 emission lag behind L1/elementwise


def _build_nc(loop_n=1):
    key = (loop_n, L2_LAG, tuple(PIPES))
    if key in _NC_CACHE:
        return _NC_CACHE[key]
    nc = bacc.Bacc("TRN2", target_bir_lowering=False, debug=False,
                   num_devices=N_CORES)
    zxa_d = nc.dram_tensor("zxa", [128, B_CORE], F32R, kind="ExternalInput").ap()
    zxb_d = nc.dram_tensor("zxb", [128, B_CORE], F32R, kind="ExternalInput").ap()
    wst_d = nc.dram_tensor("wst", [128, N_BR * 2 * 128], F32R,
                           kind="ExternalInput").ap()
    w2_d = nc.dram_tensor("w2t", [128, N_BR * 2 * 64], F16,
                          kind="ExternalInput").ap()
    w12a_d = nc.dram_tensor("w12a", [128, N_BR], F32R, kind="ExternalInput").ap()
    w12b_d = nc.dram_tensor("w12b", [128, N_BR], F32R, kind="ExternalInput").ap()
    tcol_d = nc.dram_tensor("tcol", [128, 1], F32, kind="ExternalInput").ap()
    spc_d = nc.dram_tensor("spc", [128, 1], F32, kind="ExternalInput").ap()
    out_d = nc.dram_tensor("out", [N_BR, B_CORE], F32, kind="ExternalOutput").ap()

    Silu = mybir.ActivationFunctionType.Silu
    Relu = mybir.ActivationFunctionType.Relu
    Ident = mybir.ActivationFunctionType.Identity
    Alu = mybir.AluOpType

    with tile.TileContext(nc) as tc:
        with tc.tile_pool(name="const", bufs=1) as constp, \
             tc.tile_pool(name="wstp", bufs=16) as wstp, \
             tc.tile_pool(name="tp", bufs=6) as tp, \
             tc.tile_pool(name="vp", bufs=10) as vp, \
             tc.tile_pool(name="osb", bufs=1) as osbp, \
             tc.tile_pool(name="psL1", bufs=3, space="PSUM") as psL1, \
             tc.tile_pool(name="psOut", bufs=1, space="PSUM") as psOut:

            zxa = constp.tile([128, B_CORE], F32R, tag="zxa")
            zxb = constp.tile([128, B_CORE], F32R, tag="zxb")
            w12a = constp.tile([128, N_BR], F32R, tag="w12a")
            w12b = constp.tile([128, N_BR], F32R, tag="w12b")
            tcol = constp.tile([128, 1], F32, tag="tcol")
            spc = constp.tile([128, 1], F32, tag="spc")
            w2t = constp.tile([128, N_BR * 2 * 64], F16, tag="w2t")
            wst_tiles = [wstp.tile([128, 1024], F32R, tag="wst",
                                   name=f"wst{g}") for g in range(16)]
            # One DMA queue; order so the first branches' deps land first.
            nc.sync.dma_start(zxa[:, 0:512], zxa_d[:, 0:512])
            nc.sync.dma_start(wst_tiles[0][:], wst_d[:, 0:1024])
            nc.sync.dma_start(zxa[:, 512:1024], zxa_d[:, 512:1024])
            nc.sync.dma_start(w12a[:], w12a_d[:])
            nc.sync.dma_start(w12b[:], w12b_d[:])
            nc.sync.dma_start(tcol[:], tcol_d[:])
            nc.sync.dma_start(spc[:], spc_d[:])
            nc.sync.dma_start(zxb[:], zxb_d[:])
            nc.sync.dma_start(w2t[:], w2_d[:])
            for g in range(1, 16):
                nc.sync.dma_start(wst_tiles[g][:],
                                  wst_d[:, 1024 * g:1024 * (g + 1)])

            def body(_iv=None):
                outP = psOut.tile([N_BR, B_CORE], F32, tag="out")

                def emit_linear():
                    # opens the outP accumulation (start=True); emitted a few
                    # tiles into the L1 stream so the next body's PE work
                    # doesn't block on the previous body's drain.
                    for bc in range(2):
                        sl = slice(512 * bc, 512 * (bc + 1))
                        nc.tensor.matmul(outP[:, sl], w12a[:], zxa[:, sl],
                                         start=True, stop=False,
                                         skip_group_check=True)
                        nc.tensor.matmul(outP[:, sl], w12b[:], zxb[:, sl],
                                         start=False, stop=False,
                                         skip_group_check=True)

                def emit_l2(blk, v, last):
                    for bc in range(2):
                        sl = slice(512 * bc, 512 * (bc + 1))
                        nc.tensor.matmul(outP[:, sl],
                                         w2t[:, 64 * blk:64 * (blk + 1)],
                                         v[:, sl],
                                         start=False, stop=(last and bc == 1),
                                         skip_group_check=True)

                pend = []
                nt = 0
                for br in range(N_BR):
                    mv = zxb if br == 63 else zxa
                    for hc in range(2):
                        blk = 2 * br + hc
                        pipe = PIPES[blk]
                        wg = wst_tiles[blk // 8]
                        wc = (blk % 8) * 128
                        P = psL1.tile([128, 1024], F32, tag="psl1")
                        nc.tensor.matmul(P[:, 0:512], wg[:, wc:wc + 128],
                                         mv[:, 0:512], start=True, stop=True)
                        nc.tensor.matmul(P[:, 512:1024], wg[:, wc:wc + 128],
                                         mv[:, 512:1024], start=True, stop=True)
                        v = vp.tile([128, 1024], F16, tag="v")
                        if pipe == "S":
                            nc.scalar.activation(v[:], P[:], Silu,
                                                 bias=spc[:], scale=-SP_B)
                        else:
                            t = tp.tile([128, 1024], F16, tag="t")
                            if pipe == "D":
                                nc.scalar.activation(t[:], P[:], Relu,
                                                     scale=-1.0)
                            else:  # DD
                                nc.vector.tensor_scalar(t[:], P[:], -1.0, 0.0,
                                                        Alu.mult, Alu.max)
                            nc.vector._custom_dve(PSI3_OP, out=v[:], in0=t[:],
                                                  in1=tcol[:], s0=PSI_C1,
                                                  s1=PSI_C2, imm2=PSI_C3)
                        pend.append((blk, v))
                        nt += 1
                        if nt == 6:
                            emit_linear()
                        if len(pend) > L2_LAG:
                            b0, v0 = pend.pop(0)
                            emit_l2(b0, v0, last=False)
                for i, (b0, v0) in enumerate(pend):
                    emit_l2(b0, v0, last=(i == len(pend) - 1))

                osb = osbp.tile([N_BR, B_CORE], F32, tag="osb")
                nc.scalar.activation(osb[:], outP[:], Ident)
                nc.sync.dma_start(out_d[:], osb[:])

            if isinstance(loop_n, tuple):
                n_iter, n_body = loop_n
            else:
                n_iter, n_body = loop_n, 1
            if n_iter == 1:
                for _ in range(n_body):
                    body()
            else:
                with tc.For_i(0, n_iter, 1):
                    for _ in range(n_body):
                        body()
    nc.compile()
    _NC_CACHE[key] = nc
    return nc


# ---------------- host-side prep + entry point ----------------
def _prep_shared(W1, b1, W2, b2):
    """Host-side rearrangement of the (replicated) weights."""
    W1 = np.asarray(W1, dtype=np.float32)
    b1 = np.asarray(b1, dtype=np.float32)
    W2 = np.asarray(W2, dtype=np.float32)
    b2 = np.asarray(b2, dtype=np.float32)

    wst = np.zeros((128, N_BR * 2 * 128), dtype=np.float32)
    w2t = np.zeros((128, N_BR * 2 * 64), dtype=np.float16)
    w12a = np.zeros((128, N_BR), dtype=np.float32)
    w12b = np.zeros((128, N_BR), dtype=np.float32)
    for br in range(N_BR):
        perm = np.argsort(-np.abs(W2[br]))      # big |w2| -> chunk 0
        W1p = W1[br][:, perm]
        b1p = b1[br][perm]
        W2p = W2[br][perm]
        xrow = 65 if br == 63 else 65 + br
        tgt = w12b if br == 63 else w12a
        wv = W2p.astype(np.float64)
        for hc in range(2):
            blk = 2 * br + hc
            hs = slice(128 * hc, 128 * (hc + 1))
            off = 128 * blk
            wst[0:64, off:off + 128] = W1p[1:65, hs]
            wst[64, off:off + 128] = b1p[hs]
            wst[xrow, off:off + 128] = W1p[0, hs]
            scale = 1.0 if hc == 0 else SP_A
            w2t[:, 64 * blk + br] = (scale * W2p[hs]).astype(np.float16)
        # every pipe emits only the residual; the full linear part rides here
        w12a[0:64, br] = W1p[1:65].astype(np.float64) @ wv
        tgt[xrow, br] = float(W1p[0].astype(np.float64) @ wv)
        w12a[64, br] = (float(b2[br]) + float(b1p.astype(np.float64) @ wv)
                        + SP_E * float(wv[128:].sum()))
    tcol = np.full((128, 1), PSI_T, dtype=np.float32)
    spc = np.full((128, 1), SP_C, dtype=np.float32)
    return wst, w2t, w12a, w12b, tcol, spc


def prep_core_inputs(x, z, W1, b1, W2, b2):
    x = np.asarray(x, dtype=np.float32)
    z = np.asarray(z, dtype=np.float32)
    wst, w2t, w12a, w12b, tcol, spc = _prep_shared(W1, b1, W2, b2)
    in_maps = []
    for c in range(N_CORES):
        sl = slice(c * B_CORE, (c + 1) * B_CORE)
        zxa = np.zeros((128, B_CORE), dtype=np.float32)
        zxa[0:64] = z[sl].T
        zxa[64] = 1.0
        zxa[65:128] = x[sl].T[0:63]
        zxb = np.zeros((128, B_CORE), dtype=np.float32)
        zxb[0:64] = z[sl].T
        zxb[64] = 1.0
        zxb[65] = x[sl].T[63]
        in_maps.append({"zxa": np.ascontiguousarray(zxa),
                        "zxb": np.ascontiguousarray(zxb),
                        "wst": wst, "w2t": w2t, "w12a": w12a, "w12b": w12b,
                        "tcol": tcol, "spc": spc})
    return in_maps


def kernel(x, z, W1, b1, W2, b2):
    in_maps = prep_core_inputs(x, z, W1, b1, W2, b2)
    nc = _build_nc()
    res = run_bass_kernel_spmd(nc, in_maps, list(range(N_CORES)))
    out = np.concatenate([res.results[c]["out"].T for c in range(N_CORES)],
                         axis=0)
    return np.ascontiguousarray(out).astype(np.float32)
